# revision 10
# baseline (speedup 1.0000x reference)
"""GRU decoder kernel for Trainium2 (8 NeuronCores, data-parallel over batch).

Problem (hardcoded): B=4096, T=168, D=64, H=128.

v2 design, per core (batch shard BS=512), layout [gate/hidden dim on
partitions, batch on free]:
  - Batch is split into two chunks of 256 that run the recurrence as two
    independent software-pipelined chains, so ACT/DVE/Pool/PE overlap
    instead of serializing on the per-step dependency chain.
  - feats are pre-transposed on the HOST into [128, T/2, BS] fp16 with
    even timesteps on partitions 0-63 and odd on 64-127: no on-chip
    transposes at all; plain chunked DMAs feed the feats matmuls.
  - y is not computed on-chip: h_t streams out via DMA every 4 steps and
    the host does y = wo.h_t + bo (tiny GEMV). This frees a PSUM bank and
    removes the per-step y matmul + flush op.
  - y-feedback folded into hidden matmuls exactly as v1:
      r/z: W1 = W_hh[rz] + w_y[rz] (x) wo ; n: rank-1 (wo (x) w_y_n) @ h
    Step 0 uses y0 via K=1 matmuls; biases folded per-gate into ACT bias
    APs / DVE stt scalars.
  - Per chunk-step engine split: ACT: sigmoid r, sigmoid z, tanh;
    DVE: t1=(gh_n+bhn)*r, npre=(gx_n+bn)+t1, zh=z*hmn, h'=n+zh;
    Pool (GpSimd): hmn = h_prev - n.
"""

import numpy as np

import concourse.bacc as bacc
import concourse.bass as bass
import concourse.mybir as mybir
import concourse.tile as tile
from concourse.bass_utils import run_bass_kernel_spmd

B, T, D, H = 4096, 168, 64, 128
NCORES = 8
BS = B // NCORES   # 512
CH = BS // 2       # 256 per chunk
TPC = 28           # timesteps per feats DMA chunk
NPAIR = TPC // 2

F32 = mybir.dt.float32
F16 = mybir.dt.float16
AF = mybir.ActivationFunctionType
ALU = mybir.AluOpType

# packed-constants fp16 column layout
_WYR0 = 0            # [0:1, 0:384]   w_y row (r|z|n)
_Y00 = 384           # [0:1, 384:896] y0 row
_B0 = 896            # [128, 14] fp16 = [128, 7] fp32 bias block
NPACK = 896 + 16


def build(nt=T):
    assert nt % 4 == 0
    nc = bacc.Bacc("TRN2", target_bir_lowering=False, debug=False)

    npair_tot = nt // 2
    feats = nc.declare_dram_parameter("feats", [128, npair_tot, BS], F16, isOutput=False)
    h0T_d = nc.declare_dram_parameter("h0T", [128, BS], F16, isOutput=False)
    wft_d = nc.declare_dram_parameter("wft", [128, 384], F16, isOutput=False)
    w1t_d = nc.declare_dram_parameter("w1t", [128, 256], F16, isOutput=False)
    whhnt_d = nc.declare_dram_parameter("whhnt", [128, 128], F16, isOutput=False)
    wynt_d = nc.declare_dram_parameter("wynt", [128, 128], F16, isOutput=False)
    whhrz0_d = nc.declare_dram_parameter("whhrz0", [128, 256], F16, isOutput=False)
    pack = nc.declare_dram_parameter("pack", [128, NPACK], F16, isOutput=False)

    hout = nc.declare_dram_parameter("hout", [128, nt // 4, 4 * BS], F16, isOutput=True)
    import os
    dbg = None
    if os.environ.get("KDBG"):
        dbg = nc.declare_dram_parameter("dbg", [128, 4 * BS], F32, isOutput=True)

    nchunk = (nt + TPC - 1) // TPC

    with tile.TileContext(nc) as tc:
        with (
            tc.tile_pool(name="wpool", bufs=1) as wpool,
            tc.tile_pool(name="fpool", bufs=2) as fpool,
            tc.tile_pool(name="hpool", bufs=2) as hpool,
            tc.tile_pool(name="gpool", bufs=2) as gpool,
            tc.tile_pool(name="ps_rz0", bufs=2, space="PSUM") as ps_rz0,
            tc.tile_pool(name="ps_rz1", bufs=2, space="PSUM") as ps_rz1,
            tc.tile_pool(name="ps_g0", bufs=2, space="PSUM") as ps_g0,
            tc.tile_pool(name="ps_g1", bufs=2, space="PSUM") as ps_g1,
        ):
            ps_rz = (ps_rz0, ps_rz1)
            ps_g = (ps_g0, ps_g1)

            # ---- constants ----
            pk = wpool.tile([128, NPACK], F16)
            nc.sync.dma_start(pk[:], pack[:])
            wyrow = pk[0:1, _WYR0:_WYR0 + 384]
            y0row = pk[0:1, _Y00:_Y00 + 512]
            bias = pk[:, _B0:_B0 + 14].bitcast(F32)     # [128, 7] fp32
            brz1 = bias[:, 0:2]
            brz0 = bias[:, 2:4]
            bn1 = bias[:, 4:5]
            bn0 = bias[:, 5:6]
            bhn = bias[:, 6:7]

            wft = wpool.tile([128, 384], F16)
            w1t = wpool.tile([128, 256], F16)
            whhnt = wpool.tile([128, 128], F16)
            wynt = wpool.tile([128, 128], F16)
            whhrz0 = wpool.tile([128, 256], F16)
            for sb, dr in [
                (wft, wft_d), (w1t, w1t_d), (whhnt, whhnt_d),
                (wynt, wynt_d), (whhrz0, whhrz0_d),
            ]:
                nc.sync.dma_start(sb[:], dr[:])

            h0T = wpool.tile([128, BS], F16)
            nc.sync.dma_start(h0T[:], h0T_d[:])

            # ---- feats streaming ----
            ftiles = {}

            def load_chunk(c):
                p0 = c * NPAIR
                pn = min(NPAIR, npair_tot - p0)
                ft = fpool.tile([128, NPAIR, BS], F16, tag="feat")
                nc.sync.dma_start(ft[:, :pn, :], feats[:, p0:p0 + pn, :])
                ftiles[c] = ft

            load_chunk(0)

            # ---- recurrence ----
            pend = {}

            def emit_feats(t, X):
                c = t // TPC
                par = t % 2
                tp = (t % TPC) // 2
                ft = ftiles[c]
                rhs = ft[par * 64:(par + 1) * 64, tp, X * CH:(X + 1) * CH]
                wf = wft[par * 64:(par + 1) * 64, :]
                p_rz = ps_rz[X].tile([128, 2 * CH], F32, tag="rz")
                p_g = ps_g[X].tile([128, 2 * CH], F32, tag="g")
                tpz = (par * 64, 0)
                # start=True lazily zeroes the WHOLE 2KB bank; every other
                # matmul into the bank (any column range) must use start=False.
                nc.tensor.matmul(p_rz[:, 0:CH], wf[:, 0:128], rhs,
                                 start=True, stop=False, tile_position=tpz, skip_group_check=True)
                nc.tensor.matmul(p_rz[:, CH:2 * CH], wf[:, 128:256], rhs,
                                 start=False, stop=False, tile_position=tpz, skip_group_check=True)
                nc.tensor.matmul(p_g[:, 0:CH], wf[:, 256:384], rhs,
                                 start=True, stop=False, tile_position=tpz, skip_group_check=True)
                pend[(t, X)] = (p_rz, p_g)

            def emit_hidden(t, X, hprev):
                p_rz, p_g = pend.pop((t, X))
                y0x = y0row[0:1, X * CH:(X + 1) * CH]
                if t == 0:
                    nc.tensor.matmul(p_rz[:, 0:CH], whhrz0[:, 0:128], hprev,
                                     start=False, stop=False, skip_group_check=True)
                    nc.tensor.matmul(p_rz[:, 0:CH], wyrow[0:1, 0:128], y0x,
                                     start=False, stop=True, skip_group_check=True)
                    nc.tensor.matmul(p_g[:, CH:2 * CH], whhnt[:], hprev,
                                     start=False, stop=True, skip_group_check=True)
                    nc.tensor.matmul(p_g[:, 0:CH], wyrow[0:1, 256:384], y0x,
                                     start=False, stop=True, skip_group_check=True)
                    nc.tensor.matmul(p_rz[:, CH:2 * CH], whhrz0[:, 128:256], hprev,
                                     start=False, stop=False, skip_group_check=True)
                    nc.tensor.matmul(p_rz[:, CH:2 * CH], wyrow[0:1, 128:256], y0x,
                                     start=False, stop=True, skip_group_check=True)
                else:
                    nc.tensor.matmul(p_rz[:, 0:CH], w1t[:, 0:128], hprev,
                                     start=False, stop=True, skip_group_check=True)
                    nc.tensor.matmul(p_g[:, CH:2 * CH], whhnt[:], hprev,
                                     start=False, stop=True, skip_group_check=True)
                    nc.tensor.matmul(p_g[:, 0:CH], wynt[:], hprev,
                                     start=False, stop=True, skip_group_check=True)
                    nc.tensor.matmul(p_rz[:, CH:2 * CH], w1t[:, 128:256], hprev,
                                     start=False, stop=True, skip_group_check=True)
                return p_rz, p_g

            emit_feats(0, 0)
            emit_feats(0, 1)

            hring = None
            hprev_t = h0T
            psums = {}
            gates = {}

            for t in range(nt):
                if t % TPC == 0 and t // TPC + 1 < nchunk:
                    load_chunk(t // TPC + 1)
                if t % 4 == 0:
                    hring_prev = hring
                    hring = hpool.tile([128, 4 * BS], F16, tag="hring")

                brz = brz0 if t == 0 else brz1
                bn = bn0 if t == 0 else bn1

                def hp(X):
                    if t == 0:
                        return h0T[:, X * CH:(X + 1) * CH]
                    src = hring if t % 4 != 0 else hring_prev
                    return src[:, ((t - 1) % 4) * BS + X * CH:
                               ((t - 1) % 4) * BS + (X + 1) * CH]

                # PE: hidden matmuls for t, feats for t+1
                for X in (0, 1):
                    psums[X] = emit_hidden(t, X, hp(X))
                    if t + 1 < nt:
                        emit_feats(t + 1, X)

                if t == 0 and dbg is not None:
                    dtile = wpool.tile([128, 4 * CH], F32)
                    nc.scalar.copy(dtile[:, 0:2 * CH], psums[0][0][:])
                    nc.scalar.copy(dtile[:, 2 * CH:4 * CH], psums[0][1][:])
                    nc.sync.dma_start(dbg[:, 0:4 * CH], dtile[:])

                # gate chain, chunk-interleaved
                for X in (0, 1):
                    p_rz, p_g = psums[X]
                    r16 = gpool.tile([128, CH], F16, tag=f"r{X}")
                    z16 = gpool.tile([128, CH], F16, tag=f"z{X}")
                    t1 = gpool.tile([128, CH], F16, tag=f"t1{X}")
                    npre = gpool.tile([128, CH], F16, tag=f"np{X}")
                    nc.scalar.activation(r16[:], p_rz[:, 0:CH], AF.Sigmoid,
                                         bias=brz[:, 0:1])
                    nc.scalar.activation(z16[:], p_rz[:, CH:2 * CH], AF.Sigmoid,
                                         bias=brz[:, 1:2])
                    nc.vector.scalar_tensor_tensor(
                        t1[:], p_g[:, CH:2 * CH], bhn[:, 0:1], r16[:],
                        ALU.add, ALU.mult)
                    nc.vector.scalar_tensor_tensor(
                        npre[:], p_g[:, 0:CH], bn[:, 0:1], t1[:],
                        ALU.add, ALU.add)
                    gates[X] = (r16, z16, npre)

                for X in (0, 1):
                    _, z16, npre = gates[X]
                    n16 = gpool.tile([128, CH], F16, tag=f"n{X}")
                    hmn = gpool.tile([128, CH], F16, tag=f"hm{X}")
                    zh = gpool.tile([128, CH], F16, tag=f"zh{X}")
                    hc = hring[:, (t % 4) * BS + X * CH:(t % 4) * BS + (X + 1) * CH]
                    nc.scalar.activation(n16[:], npre[:], AF.Tanh)
                    nc.gpsimd.tensor_tensor(hmn[:], hp(X), n16[:], ALU.subtract)
                    nc.vector.tensor_tensor(zh[:], z16[:], hmn[:], ALU.mult)
                    nc.vector.tensor_tensor(hc, n16[:], zh[:], ALU.add)

                if t % 4 == 3:
                    nc.sync.dma_start(hout[:, t // 4, :], hring[:])

    nc.compile()
    return nc


# -------- host-side weight prep + sharded execution --------

def _prep_aux(W_ih, W_hh, b_ih, b_hh, Wo, bo):
    W_ih = np.asarray(W_ih, np.float32)
    W_hh = np.asarray(W_hh, np.float32)
    b_ih = np.asarray(b_ih, np.float32)
    b_hh = np.asarray(b_hh, np.float32)
    wo = np.asarray(Wo, np.float32)[0]
    bo_s = float(np.asarray(bo, np.float32)[0])
    wfd = W_ih[:, :D]        # [3H, D]
    w_y = W_ih[:, D]         # [3H]

    wft = np.zeros((128, 384), np.float16)
    wft[0:64] = wfd.T.astype(np.float16)
    wft[64:128] = wfd.T.astype(np.float16)

    W1 = W_hh[0:2 * H] + np.outer(w_y[0:2 * H], wo)
    aux = dict(
        wft=wft,
        w1t=np.ascontiguousarray(W1.T.astype(np.float16)),
        whhnt=np.ascontiguousarray(W_hh[2 * H:].T.astype(np.float16)),
        wynt=np.ascontiguousarray(np.outer(wo, w_y[2 * H:]).astype(np.float16)),
        whhrz0=np.ascontiguousarray(W_hh[0:2 * H].T.astype(np.float16)),
    )

    pk = np.zeros((128, NPACK), np.float16)
    pk[0, _WYR0:_WYR0 + 384] = w_y.astype(np.float16)
    brz_base = (b_ih + b_hh)[0:2 * H]
    brz1 = np.stack(
        [brz_base[0:H] + w_y[0:H] * bo_s, brz_base[H:2 * H] + w_y[H:2 * H] * bo_s],
        axis=1).astype(np.float32)
    brz0 = np.stack([brz_base[0:H], brz_base[H:2 * H]], axis=1).astype(np.float32)
    bn1 = (b_ih[2 * H:] + w_y[2 * H:] * bo_s)[:, None].astype(np.float32)
    bn0 = b_ih[2 * H:][:, None].astype(np.float32)
    bhn = b_hh[2 * H:][:, None].astype(np.float32)
    block = np.concatenate([brz1, brz0, bn1, bn0, bhn], axis=1)  # [128, 7] f32
    pk[:, _B0:_B0 + 14] = np.ascontiguousarray(block).view(np.float16)
    aux["pack"] = pk
    aux["_wo"] = wo
    aux["_bo"] = bo_s
    return aux


def _feats_transform(fc, nt):
    """[BS, nt, D] -> [128, nt//2, BS] fp16 with even t on partitions 0:64."""
    ft = np.asarray(fc).astype(np.float16).transpose(1, 2, 0)     # [nt, D, BS]
    ft = ft.reshape(nt // 2, 2, D, BS).transpose(1, 2, 0, 3)      # [2, D, np, BS]
    return np.ascontiguousarray(ft.reshape(128, nt // 2, BS))


def _y_from_hout(hout_arr, wo, bo_s, nt):
    """[128, nt//4, 4*BS] fp16 -> y [BS, nt] fp32."""
    hm = hout_arr.reshape(128, nt, BS).astype(np.float32)
    y = np.tensordot(wo, hm, (0, 0)) + bo_s                        # [nt, BS]
    return np.ascontiguousarray(y.T)


_NC_CACHE = {}


def kernel(future_feats, h0, y0, W_ih, W_hh, b_ih, b_hh, Wo, bo):
    future_feats = np.asarray(future_feats)
    h0f = np.asarray(h0).astype(np.float16)[0]     # [B, H]
    y0f = np.asarray(y0).astype(np.float16)        # [B]

    aux = _prep_aux(W_ih, W_hh, b_ih, b_hh, Wo, bo)
    wo, bo_s = aux.pop("_wo"), aux.pop("_bo")

    if "nc" not in _NC_CACHE:
        _NC_CACHE["nc"] = build(T)
    nc = _NC_CACHE["nc"]

    in_maps = []
    for c in range(NCORES):
        sl = slice(c * BS, (c + 1) * BS)
        m = dict(aux)
        pkc = aux["pack"].copy()
        pkc[0, _Y00:_Y00 + 512] = y0f[sl]
        m["pack"] = pkc
        m["feats"] = _feats_transform(future_feats[sl], T)
        m["h0T"] = np.ascontiguousarray(h0f[sl].T)
        in_maps.append(m)

    res = run_bass_kernel_spmd(nc, in_maps, core_ids=list(range(NCORES)))
    outs = []
    for r in res.results:
        outs.append(_y_from_hout(r["hout"], wo, bo_s, T))
    return np.concatenate(outs, axis=0).astype(np.float32)
